# revision 11
# baseline (speedup 1.0000x reference)
"""GNN message-passing MLP on 8 Trainium2 NeuronCores.

Computes, for each of 2 "mc" embedding tables x (shape [N, 128]) and each of
500K edges (src, dst):
    y = relu(x[src] @ W1a + x[dst] @ W1b + b1) @ W2 + b2        # [2, E, 128]

Distribution: edge-parallel across 8 cores; node table + weights replicated
per core (no collectives).

Per-core kernel:
- The two mc tables are interleaved into one fp16 table [N, 256] (one 512B
  row per node serves both mc) and gathered with the GPSIMD dma_gather
  custom instruction in transpose mode, which lands gathered rows
  feature-major in SBUF ([128 feat, mc, edge]) - exactly the matmul layout.
- dma_gather indices are int16, so the table is addressed in 4 windows of
  25000 rows. Edges are grouped GLOBALLY (all 500K) by (src window, dst
  window) into 16 groups; each group is padded to 8*Q slots and split
  evenly across the 8 cores (Q=4096 edges per core per group), so every
  core issues exactly 32 gathers of 4096 rows. Outputs are produced in
  grouped order and inverse-permuted on the host.
- Layer 1 runs weight-stationary (hT = W1c.T @ xT in PSUM), bias+relu is
  fused on the scalar engine (b1 is per-partition in hT layout). Layer 2 is
  also weight-stationary (yT = W2c.T @ hT, accumulated over the two hidden
  chunks in PSUM), so the output stays feature-major [O, edge]; b2 (per
  partition) is added during the PSUM->SBUF copy on the vector engine, in
  fp16. y is stored transposed+fp16 ([MC, O, E_padded], 1KB contiguous per
  partition per store) - half the write traffic of the edge-major f32
  layout; the host transposes/casts back.
"""

import os
import sys

import numpy as np

for _p in ("/opt/trn_rl_repo", "/root/.axon_site/_ro/trn_rl_repo"):
    if os.path.isdir(_p) and _p not in sys.path:
        sys.path.insert(0, _p)

import concourse.bass as bass
import concourse.mybir as mybir
import concourse.tile as tile
from concourse import bacc
from concourse.bass_utils import run_bass_kernel_spmd

# Problem constants (hardcoded per harness contract).
N_NODES = 100000
E_TOTAL = 500000
D = 128          # input feature dim
H = 256          # hidden dim
O = 128          # output dim
MC = 2           # number of embedding tables
CORES = 8
P = 128

# Windowed gather layout.
WN = 25000                   # nodes per index window (int16-addressable)
WC = 4                       # windows
NG = WC * WC                 # (src window, dst window) groups
Q = 4096                     # per-core per-group quota (global mean 3906)
EC_DEV = NG * Q              # padded edges per core (65536)
GSLOT = NG * CORES * Q       # global padded slots (524288)
SUBW = 512                   # compute batch width (edges per L1 matmul)

_CACHE = {}
_last_in_maps = None


def _build(repeats=1, queues=1, gather_only=False, compute=True,
           single_packet=False, xg_bufs=2, scratch=16384, chunk=Q):
    f16 = mybir.dt.float16
    f32 = mybir.dt.float32
    i16 = mybir.dt.int16

    idx_cols_per_group = 2 * (Q // 16)           # src + dst, wrapped by 16
    idx_cols = NG * idx_cols_per_group           # 8192

    nc = bacc.Bacc("TRN2", target_bir_lowering=False, num_devices=CORES,
                   num_swdge_queues=queues,
                   dynamic_dma_scratch_size=scratch)
    tab = nc.declare_dram_parameter("tab", [N_NODES, MC * D], f16, isOutput=False)
    idx = nc.declare_dram_parameter("idx", [P, idx_cols], i16, isOutput=False)
    w1 = nc.declare_dram_parameter("w1", [2, D, H], f16, isOutput=False)
    w2 = nc.declare_dram_parameter("w2", [H // P, P, O], f16, isOutput=False)
    b1 = nc.declare_dram_parameter("b1", [H // P, P], f32, isOutput=False)
    b2 = nc.declare_dram_parameter("b2", [P, 1], f32, isOutput=False)
    y = nc.declare_dram_parameter("y", [MC, O, EC_DEV], f16, isOutput=True)

    relu = mybir.ActivationFunctionType.Relu

    with tile.TileContext(nc) as tc:
        with (
            tc.tile_pool(name="const", bufs=1) as cpool,
            tc.tile_pool(name="xg", bufs=xg_bufs) as xgpool,
            tc.tile_pool(name="ht", bufs=3) as htpool,
            tc.tile_pool(name="yo", bufs=3) as yopool,
            tc.tile_pool(name="ph", bufs=2, space="PSUM") as phpool,
            tc.tile_pool(name="py", bufs=2, space="PSUM") as pypool,
        ):
            w1_sb = cpool.tile([P, 2, H], f16)       # [d, a/b, h]
            nc.sync.dma_start(w1_sb[:], w1.rearrange("a d h -> d a h"))
            w2_sb = cpool.tile([P, H // P, O], f16)  # [h_in_chunk, chunk, o]
            nc.sync.dma_start(w2_sb[:], w2.rearrange("c h o -> h c o"))
            b1_sb = cpool.tile([P, H // P], f32)
            nc.sync.dma_start(b1_sb[:], b1.rearrange("c p -> p c"))
            b2_sb = cpool.tile([P, 1], f32)
            nc.sync.dma_start(b2_sb[:], b2[:])
            ix_all = cpool.tile([P, idx_cols], i16)
            nc.sync.dma_start(ix_all[:], idx[:])

            nj = Q // chunk

            def one_pass():
                for g in range(NG):
                    ws, wd = g // WC, g % WC
                    icol = g * idx_cols_per_group
                    xs_t, xd_t = [], []
                    for j in range(nj):
                        c0 = icol + j * (chunk // 16)
                        xs = xgpool.tile([P, MC, chunk], f16, tag=f"xs{j}")
                        nc.gpsimd.dma_gather(
                            out_ap=xs[:],
                            in_ap=tab[ws * WN:(ws + 1) * WN, :],
                            idxs_ap=ix_all[:, c0:c0 + chunk // 16],
                            num_idxs=chunk,
                            num_idxs_reg=chunk,
                            elem_size=MC * D,
                            transpose=True,
                            single_packet=single_packet,
                            queue_num=(2 * (g * nj + j)) % queues,
                        )
                        xs_t.append(xs)
                        c0 = icol + Q // 16 + j * (chunk // 16)
                        xd = xgpool.tile([P, MC, chunk], f16, tag=f"xd{j}")
                        nc.gpsimd.dma_gather(
                            out_ap=xd[:],
                            in_ap=tab[wd * WN:(wd + 1) * WN, :],
                            idxs_ap=ix_all[:, c0:c0 + chunk // 16],
                            num_idxs=chunk,
                            num_idxs_reg=chunk,
                            elem_size=MC * D,
                            transpose=True,
                            single_packet=single_packet,
                            queue_num=(2 * (g * nj + j) + 1) % queues,
                        )
                        xd_t.append(xd)
                    if gather_only:
                        continue
                    for o_ in range(0, Q, SUBW):
                        xs = xs_t[o_ // chunk]
                        xd = xd_t[o_ // chunk]
                        oc = o_ % chunk
                        for mc in range(MC):
                            hts = []
                            for c in range(H // P):
                                ph = phpool.tile([P, SUBW], f32, tag=f"ph{c}")
                                if compute:
                                    nc.tensor.matmul(
                                        ph[:],
                                        lhsT=w1_sb[:, 0, c * P:(c + 1) * P],
                                        rhs=xs[:, mc, oc:oc + SUBW],
                                        start=True, stop=False,
                                    )
                                    nc.tensor.matmul(
                                        ph[:],
                                        lhsT=w1_sb[:, 1, c * P:(c + 1) * P],
                                        rhs=xd[:, mc, oc:oc + SUBW],
                                        start=False, stop=True,
                                    )
                                ht = htpool.tile([P, SUBW], f16, tag=f"ht{c}")
                                nc.scalar.activation(
                                    ht[:], ph[:], relu,
                                    bias=b1_sb[:, c:c + 1],
                                )
                                hts.append(ht)

                            py = pypool.tile([P, SUBW], f32, tag="py")
                            nc.tensor.matmul(
                                py[:], lhsT=w2_sb[:, 0, :], rhs=hts[0][:],
                                start=True, stop=False,
                            )
                            nc.tensor.matmul(
                                py[:], lhsT=w2_sb[:, 1, :], rhs=hts[1][:],
                                start=False, stop=True,
                            )
                            yo = yopool.tile([P, SUBW], f16, tag="yo")
                            nc.vector.tensor_scalar_add(yo[:], py[:],
                                                        b2_sb[:, 0:1])
                            nc.sync.dma_start(
                                y[mc, :, g * Q + o_:g * Q + o_ + SUBW],
                                yo[:],
                            )
                if gather_only:
                    # Touch y once so the output tensor is produced.
                    yo = yopool.tile([P, SUBW], f16, tag="yo")
                    nc.vector.memset(yo[:], 0.0)
                    nc.sync.dma_start(y[0, :, 0:SUBW], yo[:])

            hw_loop = os.environ.get("GNN_HW_LOOP", "1") == "1"
            if repeats > 1 and hw_loop:
                with tc.For_i(0, repeats):
                    one_pass()
            else:
                for _ in range(repeats):
                    one_pass()

    nc.compile()
    return nc


def _get_program(repeats=1):
    queues = int(os.environ.get("GNN_QUEUES", "1"))
    single_packet = os.environ.get("GNN_SINGLE_PACKET", "0") == "1"
    key = (repeats, queues, single_packet)
    if key not in _CACHE:
        _CACHE[key] = _build(repeats, queues=queues,
                             single_packet=single_packet)
    return _CACHE[key]


def _wrap_idx(flat):
    """[n*16k] int -> [128, n/16] int16, wrapped by 16, replicated 8x."""
    w = flat.reshape(-1, 16).T.astype(np.int16)      # [16, n/16]
    return np.tile(w, (8, 1))                        # [128, n/16]


def _prep_edges(src, dst):
    """Group ALL edges by (src window, dst window); split groups across cores.

    Returns (idx_arrs: list of 8 [128, idx_cols] int16, perm) where
    perm[s] = original edge position of global padded slot s (-1 = padding).
    Group g occupies global slots [g*8Q, (g+1)*8Q); core c takes the
    sub-range [g*8Q + c*Q, g*8Q + (c+1)*Q).
    """
    ws = src // WN
    wd = dst // WN
    g = ws * WC + wd
    order = np.argsort(g, kind="stable")
    counts = np.bincount(g, minlength=NG)
    if counts.max() > CORES * Q:
        raise ValueError(f"group overflow: {counts.max()} > {CORES * Q}")

    perm = np.full(GSLOT, -1, dtype=np.int64)
    src_p = np.zeros(GSLOT, dtype=np.int64)
    dst_p = np.zeros(GSLOT, dtype=np.int64)
    pos = 0
    for gi in range(NG):
        n = counts[gi]
        sel = order[pos:pos + n]
        pos += n
        base = gi * CORES * Q
        perm[base:base + n] = sel
        w_s, w_d = gi // WC, gi % WC
        src_p[base:base + n] = src[sel] - w_s * WN
        dst_p[base:base + n] = dst[sel] - w_d * WN

    idx_arrs = []
    for c in range(CORES):
        cols = []
        for gi in range(NG):
            base = gi * CORES * Q + c * Q
            cols.append(_wrap_idx(src_p[base:base + Q]))
            cols.append(_wrap_idx(dst_p[base:base + Q]))
        idx_arrs.append(np.ascontiguousarray(np.concatenate(cols, axis=1)))
    return idx_arrs, perm


def kernel(edge_index, mc_embeddings, W1, b1, W2, b2):
    nc = _get_program(1)

    edge_index = np.asarray(edge_index)
    mc_embeddings = np.asarray(mc_embeddings, dtype=np.float32)
    W1 = np.asarray(W1, dtype=np.float32)
    b1 = np.asarray(b1, dtype=np.float32)
    W2 = np.asarray(W2, dtype=np.float32)
    b2 = np.asarray(b2, dtype=np.float32)

    # mc-interleaved fp16 node table: row n = [x0[n] | x1[n]].
    tab = np.ascontiguousarray(
        mc_embeddings.transpose(1, 0, 2).reshape(N_NODES, MC * D)
    ).astype(np.float16)
    w1_in = np.ascontiguousarray(W1.reshape(2, D, H)).astype(np.float16)
    w2_in = np.ascontiguousarray(W2.reshape(H // P, P, O)).astype(np.float16)
    b1_in = np.ascontiguousarray(b1.reshape(H // P, P)).astype(np.float32)
    b2_in = np.ascontiguousarray(b2.reshape(P, 1)).astype(np.float32)

    idx64 = edge_index.astype(np.int64)
    idx_arrs, perm = _prep_edges(idx64[0], idx64[1])
    in_maps = [
        {
            "tab": tab,
            "idx": idx_arrs[c],
            "w1": w1_in,
            "w2": w2_in,
            "b1": b1_in,
            "b2": b2_in,
        }
        for c in range(CORES)
    ]

    global _last_in_maps
    _last_in_maps = in_maps
    res = run_bass_kernel_spmd(nc, in_maps, list(range(CORES)))

    # y_all[c, mc, f, g*Q + i]  <->  global slot g*8Q + c*Q + i.
    y_all = np.stack([res.results[c]["y"] for c in range(CORES)])
    glob = (
        y_all.reshape(CORES, MC, O, NG, Q)
        .transpose(1, 2, 3, 0, 4)
        .reshape(MC, O, GSLOT)
    )
    valid = perm >= 0
    out = np.empty((MC, E_TOTAL, O), dtype=np.float32)
    out[:, perm[valid], :] = glob[:, :, valid].transpose(0, 2, 1)
    return out


# revision 18
# speedup vs baseline: 1.6211x; 1.6211x over previous
"""GNN message-passing MLP on 8 Trainium2 NeuronCores.

Computes, for each of 2 "mc" embedding tables x (shape [N, 128]) and each of
500K edges (src, dst):
    y = relu(x[src] @ W1a + x[dst] @ W1b + b1) @ W2 + b2        # [2, E, 128]

Distribution: edge-parallel across 8 cores; node table + weights replicated
per core (no collectives).

Per-core kernel:
- The two mc tables are interleaved into one fp16 table [N, 256] (one 512B
  row per node serves both mc) and gathered with the GPSIMD dma_gather
  custom instruction in transpose mode, which lands gathered rows
  feature-major in SBUF ([128 feat, mc, edge]) - exactly the matmul layout.
- dma_gather indices are int16, so the table is addressed in 4 windows of
  25000 rows. Edges are grouped GLOBALLY (all 500K) by (src window, dst
  window) into 16 groups; each group is padded to 8*Q slots and split
  evenly across the 8 cores (Q=4096 edges per core per group), so every
  core issues exactly 32 gathers of 4096 rows. Outputs are produced in
  grouped order and inverse-permuted on the host.
- Layer 1 runs weight-stationary (hT = W1c.T @ xT in PSUM), bias+relu is
  fused on the scalar engine (b1 is per-partition in hT layout). Layer 2 is
  also weight-stationary (yT = W2c.T @ hT, accumulated over the two hidden
  chunks in PSUM), so the output stays feature-major [O, edge]; b2 (per
  partition) is added during the PSUM->SBUF copy on the vector engine, in
  fp16. y is stored transposed+fp16 ([MC, O, E_padded], 1KB contiguous per
  partition per store) - half the write traffic of the edge-major f32
  layout; the host transposes/casts back.
"""

import os
import sys

import numpy as np

for _p in ("/opt/trn_rl_repo", "/root/.axon_site/_ro/trn_rl_repo"):
    if os.path.isdir(_p) and _p not in sys.path:
        sys.path.insert(0, _p)

import concourse.bass as bass
import concourse.mybir as mybir
import concourse.tile as tile
from concourse import bacc
from concourse.bass_utils import run_bass_kernel_spmd

# Problem constants (hardcoded per harness contract).
N_NODES = 100000
E_TOTAL = 500000
D = 128          # input feature dim
H = 256          # hidden dim
O = 128          # output dim
MC = 2           # number of embedding tables
CORES = 8
P = 128

# Windowed gather layout.
WN = 25000                   # nodes per index window (int16-addressable)
WC = 4                       # windows
NG = WC * WC                 # (src window, dst window) groups
Q = 4096                     # per-core per-group quota (global mean 3906)
EC_DEV = NG * Q              # padded edges per core (65536)
GSLOT = NG * CORES * Q       # global padded slots (524288)
SUBW = 512                   # compute batch width (edges per L1 matmul)

_CACHE = {}
_last_in_maps = None


def _build(repeats=1, queues=1, gather_only=False, compute=True,
           single_packet=False, xg_bufs=2, scratch=16384, chunk=Q,
           dt8=False, elem_mult=1):
    f16 = mybir.dt.float16
    f32 = mybir.dt.float32
    f8 = mybir.dt.float8e4
    i16 = mybir.dt.int16
    xdt = f8 if dt8 else f16

    idx_cols_per_group = 2 * (Q // 16)           # src + dst, wrapped by 16
    idx_cols = NG * idx_cols_per_group           # 8192

    nc = bacc.Bacc("TRN2", target_bir_lowering=False, num_devices=CORES,
                   num_swdge_queues=queues,
                   dynamic_dma_scratch_size=scratch)
    tab = nc.declare_dram_parameter("tab", [N_NODES, elem_mult * MC * D],
                                    xdt, isOutput=False)
    idx = nc.declare_dram_parameter("idx", [P, idx_cols], i16, isOutput=False)
    if dt8:
        # [d, ktile(src/dst), h] fp8, prearranged on host for DoubleRow.
        w1 = nc.declare_dram_parameter("w1", [D, 2, H], f8, isOutput=False)
    else:
        w1 = nc.declare_dram_parameter("w1", [2, D, H], f16, isOutput=False)
    w2 = nc.declare_dram_parameter("w2", [H // P, P, O], f16, isOutput=False)
    b1 = nc.declare_dram_parameter("b1", [H // P, P], f32, isOutput=False)
    b2 = nc.declare_dram_parameter("b2", [P, 1], f32, isOutput=False)
    y = nc.declare_dram_parameter("y", [MC, O, EC_DEV], f16, isOutput=True)

    relu = mybir.ActivationFunctionType.Relu
    drow = mybir.MatmulPerfMode.DoubleRow
    nc._gnn_dt8 = dt8

    with tile.TileContext(nc) as tc:
        with (
            tc.tile_pool(name="const", bufs=1) as cpool,
            tc.tile_pool(name="xg", bufs=xg_bufs) as xgpool,
            tc.tile_pool(name="ht", bufs=3) as htpool,
            tc.tile_pool(name="yo", bufs=3) as yopool,
            tc.tile_pool(name="ph", bufs=2, space="PSUM") as phpool,
            tc.tile_pool(name="py", bufs=2, space="PSUM") as pypool,
        ):
            if dt8:
                w1_sb = cpool.tile([P, 2, H], f8)    # [d, ktile, h]
                nc.sync.dma_start(w1_sb[:], w1[:])
            else:
                w1_sb = cpool.tile([P, 2, H], f16)   # [d, a/b, h]
                nc.sync.dma_start(w1_sb[:], w1.rearrange("a d h -> d a h"))
            w2_sb = cpool.tile([P, H // P, O], f16)  # [h_in_chunk, chunk, o]
            nc.sync.dma_start(w2_sb[:], w2.rearrange("c h o -> h c o"))
            b1_sb = cpool.tile([P, H // P], f32)
            nc.sync.dma_start(b1_sb[:], b1.rearrange("c p -> p c"))
            b2_sb = cpool.tile([P, 1], f32)
            nc.sync.dma_start(b2_sb[:], b2[:])
            ix_all = cpool.tile([P, idx_cols], i16)
            nc.sync.dma_start(ix_all[:], idx[:])

            nj = Q // chunk

            def one_pass():
                for g in range(NG):
                    ws, wd = g // WC, g % WC
                    icol = g * idx_cols_per_group
                    xs_t, xd_t = [], []
                    for j in range(nj):
                        cs = icol + j * (chunk // 16)
                        cd = icol + Q // 16 + j * (chunk // 16)
                        if dt8:
                            # One tile holds both endpoints as DoubleRow
                            # k-tiles: [p, ktile, a, e] fp8 (bytes per
                            # (p, ktile): edge e at (2e, 2e+1) = (mc0, mc1)).
                            xb = xgpool.tile([P, 2, 2, chunk], f8,
                                             tag=f"x{j}")
                            srcs = (xb[:, 0], xb[:, 1])
                        else:
                            xs = xgpool.tile([P, elem_mult * MC, chunk], f16,
                                             tag=f"xs{j}")
                            xd = xgpool.tile([P, elem_mult * MC, chunk], f16,
                                             tag=f"xd{j}")
                            srcs = (xs[:], xd[:])
                        for k, (win, c0) in enumerate(((ws, cs), (wd, cd))):
                            nc.gpsimd.dma_gather(
                                out_ap=srcs[k],
                                in_ap=tab[win * WN:(win + 1) * WN, :],
                                idxs_ap=ix_all[:, c0:c0 + chunk // 16],
                                num_idxs=chunk,
                                num_idxs_reg=chunk,
                                elem_size=elem_mult * MC * D,
                                transpose=True,
                                single_packet=single_packet,
                                queue_num=(2 * (g * nj + j) + k) % queues,
                            )
                        if dt8:
                            xv = (
                                xb[:]
                                .rearrange("p k a e -> p k (a e)")
                                .rearrange("p k (e m) -> p k e m", m=MC)
                            )
                            xs_t.append(xv)
                            xd_t.append(xv)
                        else:
                            xs_t.append(xs)
                            xd_t.append(xd)
                    if gather_only:
                        continue
                    for o_ in range(0, Q, SUBW):
                        xs = xs_t[o_ // chunk]
                        xd = xd_t[o_ // chunk]
                        oc = o_ % chunk
                        for mc in range(MC):
                            hts = []
                            for c in range(H // P):
                                ph = phpool.tile([P, SUBW], f32, tag=f"ph{c}")
                                if compute and dt8:
                                    nc.tensor.matmul(
                                        ph[:],
                                        lhsT=w1_sb[:, :, c * P:(c + 1) * P],
                                        rhs=xs[:, :, oc:oc + SUBW, mc],
                                        start=True, stop=True,
                                        perf_mode=drow,
                                    )
                                elif compute:
                                    nc.tensor.matmul(
                                        ph[:],
                                        lhsT=w1_sb[:, 0, c * P:(c + 1) * P],
                                        rhs=xs[:, mc, oc:oc + SUBW],
                                        start=True, stop=False,
                                    )
                                    nc.tensor.matmul(
                                        ph[:],
                                        lhsT=w1_sb[:, 1, c * P:(c + 1) * P],
                                        rhs=xd[:, mc, oc:oc + SUBW],
                                        start=False, stop=True,
                                    )
                                ht = htpool.tile([P, SUBW], f16, tag=f"ht{c}")
                                nc.scalar.activation(
                                    ht[:], ph[:], relu,
                                    bias=b1_sb[:, c:c + 1],
                                )
                                hts.append(ht)

                            py = pypool.tile([P, SUBW], f32, tag="py")
                            nc.tensor.matmul(
                                py[:], lhsT=w2_sb[:, 0, :], rhs=hts[0][:],
                                start=True, stop=False,
                            )
                            nc.tensor.matmul(
                                py[:], lhsT=w2_sb[:, 1, :], rhs=hts[1][:],
                                start=False, stop=True,
                            )
                            yo = yopool.tile([P, SUBW], f16, tag="yo")
                            nc.vector.tensor_scalar_add(yo[:], py[:],
                                                        b2_sb[:, 0:1])
                            nc.sync.dma_start(
                                y[mc, :, g * Q + o_:g * Q + o_ + SUBW],
                                yo[:],
                            )
                if gather_only:
                    # Touch y once so the output tensor is produced.
                    yo = yopool.tile([P, SUBW], f16, tag="yo")
                    nc.vector.memset(yo[:], 0.0)
                    nc.sync.dma_start(y[0, :, 0:SUBW], yo[:])

            hw_loop = os.environ.get("GNN_HW_LOOP", "1") == "1"
            if repeats > 1 and hw_loop:
                with tc.For_i(0, repeats):
                    one_pass()
            else:
                for _ in range(repeats):
                    one_pass()

    nc.compile()
    return nc


def _get_program(repeats=1):
    queues = int(os.environ.get("GNN_QUEUES", "1"))
    single_packet = os.environ.get("GNN_SINGLE_PACKET", "0") == "1"
    chunk = int(os.environ.get("GNN_CHUNK", "1024"))
    xg_bufs = int(os.environ.get("GNN_XGBUFS", "2"))
    dt8 = os.environ.get("GNN_DT8", "1") == "1"
    key = (repeats, queues, single_packet, chunk, xg_bufs, dt8)
    if key not in _CACHE:
        _CACHE[key] = _build(repeats, queues=queues,
                             single_packet=single_packet, chunk=chunk,
                             xg_bufs=xg_bufs, dt8=dt8)
    return _CACHE[key]


def _wrap_idx(flat):
    """[n*16k] int -> [128, n/16] int16, wrapped by 16, replicated 8x."""
    w = flat.reshape(-1, 16).T.astype(np.int16)      # [16, n/16]
    return np.tile(w, (8, 1))                        # [128, n/16]


def _prep_edges(src, dst):
    """Group ALL edges by (src window, dst window); split groups across cores.

    Returns (idx_arrs: list of 8 [128, idx_cols] int16, perm) where
    perm[s] = original edge position of global padded slot s (-1 = padding).
    Group g occupies global slots [g*8Q, (g+1)*8Q); core c takes the
    sub-range [g*8Q + c*Q, g*8Q + (c+1)*Q).
    """
    ws = src // WN
    wd = dst // WN
    g = ws * WC + wd
    order = np.argsort(g, kind="stable")
    counts = np.bincount(g, minlength=NG)
    if counts.max() > CORES * Q:
        raise ValueError(f"group overflow: {counts.max()} > {CORES * Q}")

    perm = np.full(GSLOT, -1, dtype=np.int64)
    src_p = np.zeros(GSLOT, dtype=np.int64)
    dst_p = np.zeros(GSLOT, dtype=np.int64)
    pos = 0
    for gi in range(NG):
        n = counts[gi]
        sel = order[pos:pos + n]
        pos += n
        base = gi * CORES * Q
        perm[base:base + n] = sel
        w_s, w_d = gi // WC, gi % WC
        src_p[base:base + n] = src[sel] - w_s * WN
        dst_p[base:base + n] = dst[sel] - w_d * WN

    idx_arrs = []
    for c in range(CORES):
        cols = []
        for gi in range(NG):
            base = gi * CORES * Q + c * Q
            cols.append(_wrap_idx(src_p[base:base + Q]))
            cols.append(_wrap_idx(dst_p[base:base + Q]))
        idx_arrs.append(np.ascontiguousarray(np.concatenate(cols, axis=1)))
    return idx_arrs, perm


def kernel(edge_index, mc_embeddings, W1, b1, W2, b2):
    nc = _get_program(1)

    edge_index = np.asarray(edge_index)
    mc_embeddings = np.asarray(mc_embeddings, dtype=np.float32)
    W1 = np.asarray(W1, dtype=np.float32)
    b1 = np.asarray(b1, dtype=np.float32)
    W2 = np.asarray(W2, dtype=np.float32)
    b2 = np.asarray(b2, dtype=np.float32)

    dt8 = getattr(nc, "_gnn_dt8", False)
    if dt8:
        f8np = mybir.dt.np(mybir.dt.float8e4)
        # Feature-major mc interleave: row n = [f0mc0, f0mc1, f1mc0, ...].
        tab = np.ascontiguousarray(
            mc_embeddings.transpose(1, 2, 0).reshape(N_NODES, MC * D)
        ).astype(f8np)
        # [d, ktile(src/dst), h] for the DoubleRow lhsT.
        w1_in = np.ascontiguousarray(
            W1.reshape(2, D, H).transpose(1, 0, 2)
        ).astype(f8np)
    else:
        # mc-interleaved fp16 node table: row n = [x0[n] | x1[n]].
        tab = np.ascontiguousarray(
            mc_embeddings.transpose(1, 0, 2).reshape(N_NODES, MC * D)
        ).astype(np.float16)
        w1_in = np.ascontiguousarray(W1.reshape(2, D, H)).astype(np.float16)
    w2_in = np.ascontiguousarray(W2.reshape(H // P, P, O)).astype(np.float16)
    b1_in = np.ascontiguousarray(b1.reshape(H // P, P)).astype(np.float32)
    b2_in = np.ascontiguousarray(b2.reshape(P, 1)).astype(np.float32)

    idx64 = edge_index.astype(np.int64)
    idx_arrs, perm = _prep_edges(idx64[0], idx64[1])
    in_maps = [
        {
            "tab": tab,
            "idx": idx_arrs[c],
            "w1": w1_in,
            "w2": w2_in,
            "b1": b1_in,
            "b2": b2_in,
        }
        for c in range(CORES)
    ]

    global _last_in_maps
    _last_in_maps = in_maps
    res = run_bass_kernel_spmd(nc, in_maps, list(range(CORES)))

    # y_all[c, mc, f, g*Q + i]  <->  global slot g*8Q + c*Q + i.
    y_all = np.stack([res.results[c]["y"] for c in range(CORES)])
    glob = (
        y_all.reshape(CORES, MC, O, NG, Q)
        .transpose(1, 2, 3, 0, 4)
        .reshape(MC, O, GSLOT)
    )
    valid = perm >= 0
    out = np.empty((MC, E_TOTAL, O), dtype=np.float32)
    out[:, perm[valid], :] = glob[:, :, valid].transpose(0, 2, 1)
    return out


# revision 20
# speedup vs baseline: 1.9648x; 1.2120x over previous
"""GNN message-passing MLP on 8 Trainium2 NeuronCores.

Computes, for each of 2 "mc" embedding tables x (shape [N, 128]) and each of
500K edges (src, dst):
    y = relu(x[src] @ W1a + x[dst] @ W1b + b1) @ W2 + b2        # [2, E, 128]

Distribution: edge-parallel across 8 cores; node table + weights replicated
per core (no collectives).

Per-core kernel:
- The two mc tables are interleaved into one fp16 table [N, 256] (one 512B
  row per node serves both mc) and gathered with the GPSIMD dma_gather
  custom instruction in transpose mode, which lands gathered rows
  feature-major in SBUF ([128 feat, mc, edge]) - exactly the matmul layout.
- dma_gather indices are int16, so the table is addressed in 4 windows of
  25000 rows. Edges are grouped GLOBALLY (all 500K) by (src window, dst
  window) into 16 groups; each group is padded to 8*Q slots and split
  evenly across the 8 cores (Q=4096 edges per core per group). Gathers are
  issued in chunks of 1024 rows (128 gathers/core/pass): the gather is
  DESCRIPTOR-LATENCY-bound (~140ns per 512B row per DMA engine; 2x-bytes
  experiment shows +9% time, so not bandwidth-bound), and smaller chunks
  keep more gathers resident in the 1024-desc/engine SWDGE ring, nearly
  doubling throughput vs 4096-row chunks. Outputs are produced in grouped
  order and inverse-permuted on the host.
- Known dead ends (measured): num_swdge_queues=2 gives wrong results (queue-1
  completions not awaited; ucode-level bug) and is slower anyway;
  dynamic_dma_scratch_size=65536 crashes the device (NRT unrecoverable);
  single_packet=True desyncs the mesh; fp8 table+DoubleRow L1 is accuracy-
  infeasible (3.7% rel err; fp8 quantization error does not average down)
  and no faster (latency-bound gather).
- Layer 1 runs weight-stationary (hT = W1c.T @ xT in PSUM), bias+relu is
  fused on the scalar engine (b1 is per-partition in hT layout). Layer 2 is
  also weight-stationary (yT = W2c.T @ hT, accumulated over the two hidden
  chunks in PSUM), so the output stays feature-major [O, edge]; b2 (per
  partition) is added during the PSUM->SBUF copy on the vector engine, in
  fp16. y is stored transposed+fp16 ([MC, O, E_padded], 1KB contiguous per
  partition per store) - half the write traffic of the edge-major f32
  layout; the host transposes/casts back.
"""

import os
import sys

import numpy as np

for _p in ("/opt/trn_rl_repo", "/root/.axon_site/_ro/trn_rl_repo"):
    if os.path.isdir(_p) and _p not in sys.path:
        sys.path.insert(0, _p)

import concourse.bass as bass
import concourse.mybir as mybir
import concourse.tile as tile
from concourse import bacc
from concourse.bass_utils import run_bass_kernel_spmd

# Problem constants (hardcoded per harness contract).
N_NODES = 100000
E_TOTAL = 500000
D = 128          # input feature dim
H = 256          # hidden dim
O = 128          # output dim
MC = 2           # number of embedding tables
CORES = 8
P = 128

# Windowed gather layout.
WN = 25000                   # nodes per index window (int16-addressable)
WC = 4                       # windows
NG = WC * WC                 # (src window, dst window) groups
Q = 4096                     # per-core per-group quota (global mean 3906)
EC_DEV = NG * Q              # padded edges per core (65536)
GSLOT = NG * CORES * Q       # global padded slots (524288)
SUBW = 512                   # compute batch width (edges per L1 matmul)

_CACHE = {}
_last_in_maps = None


def _build(repeats=1, queues=1, gather_only=False, compute=True,
           single_packet=False, xg_bufs=2, scratch=16384, chunk=Q,
           dt8=False, elem_mult=1):
    f16 = mybir.dt.float16
    f32 = mybir.dt.float32
    f8 = mybir.dt.float8e4
    i16 = mybir.dt.int16
    xdt = f8 if dt8 else f16

    idx_cols_per_group = 2 * (Q // 16)           # src + dst, wrapped by 16
    idx_cols = NG * idx_cols_per_group           # 8192

    nc = bacc.Bacc("TRN2", target_bir_lowering=False, num_devices=CORES,
                   num_swdge_queues=queues,
                   dynamic_dma_scratch_size=scratch)
    tab = nc.declare_dram_parameter("tab", [N_NODES, elem_mult * MC * D],
                                    xdt, isOutput=False)
    idx = nc.declare_dram_parameter("idx", [P, idx_cols], i16, isOutput=False)
    if dt8:
        # [d, ktile(src/dst), h] fp8, prearranged on host for DoubleRow.
        w1 = nc.declare_dram_parameter("w1", [D, 2, H], f8, isOutput=False)
    else:
        w1 = nc.declare_dram_parameter("w1", [2, D, H], f16, isOutput=False)
    w2 = nc.declare_dram_parameter("w2", [H // P, P, O], f16, isOutput=False)
    b1 = nc.declare_dram_parameter("b1", [H // P, P], f32, isOutput=False)
    b2 = nc.declare_dram_parameter("b2", [P, 1], f32, isOutput=False)
    y = nc.declare_dram_parameter("y", [MC, O, EC_DEV], f16, isOutput=True)

    relu = mybir.ActivationFunctionType.Relu
    drow = mybir.MatmulPerfMode.DoubleRow
    nc._gnn_dt8 = dt8

    with tile.TileContext(nc) as tc:
        with (
            tc.tile_pool(name="const", bufs=1) as cpool,
            tc.tile_pool(name="xg", bufs=xg_bufs) as xgpool,
            tc.tile_pool(name="ht", bufs=3) as htpool,
            tc.tile_pool(name="yo", bufs=3) as yopool,
            tc.tile_pool(name="ph", bufs=2, space="PSUM") as phpool,
            tc.tile_pool(name="py", bufs=2, space="PSUM") as pypool,
        ):
            if dt8:
                w1_sb = cpool.tile([P, 2, H], f8)    # [d, ktile, h]
                nc.sync.dma_start(w1_sb[:], w1[:])
            else:
                w1_sb = cpool.tile([P, 2, H], f16)   # [d, a/b, h]
                nc.sync.dma_start(w1_sb[:], w1.rearrange("a d h -> d a h"))
            w2_sb = cpool.tile([P, H // P, O], f16)  # [h_in_chunk, chunk, o]
            nc.sync.dma_start(w2_sb[:], w2.rearrange("c h o -> h c o"))
            b1_sb = cpool.tile([P, H // P], f32)
            nc.sync.dma_start(b1_sb[:], b1.rearrange("c p -> p c"))
            b2_sb = cpool.tile([P, 1], f32)
            nc.sync.dma_start(b2_sb[:], b2[:])
            ix_all = cpool.tile([P, idx_cols], i16)
            nc.sync.dma_start(ix_all[:], idx[:])

            nj = Q // chunk

            def one_pass():
                for g in range(NG):
                    ws, wd = g // WC, g % WC
                    icol = g * idx_cols_per_group
                    xs_t, xd_t = [], []
                    for j in range(nj):
                        cs = icol + j * (chunk // 16)
                        cd = icol + Q // 16 + j * (chunk // 16)
                        if dt8:
                            # One tile holds both endpoints as DoubleRow
                            # k-tiles: [p, ktile, a, e] fp8 (bytes per
                            # (p, ktile): edge e at (2e, 2e+1) = (mc0, mc1)).
                            xb = xgpool.tile([P, 2, 2, chunk], f8,
                                             tag=f"x{j}")
                            srcs = (xb[:, 0], xb[:, 1])
                        else:
                            xs = xgpool.tile([P, elem_mult * MC, chunk], f16,
                                             tag=f"xs{j}")
                            xd = xgpool.tile([P, elem_mult * MC, chunk], f16,
                                             tag=f"xd{j}")
                            srcs = (xs[:], xd[:])
                        for k, (win, c0) in enumerate(((ws, cs), (wd, cd))):
                            nc.gpsimd.dma_gather(
                                out_ap=srcs[k],
                                in_ap=tab[win * WN:(win + 1) * WN, :],
                                idxs_ap=ix_all[:, c0:c0 + chunk // 16],
                                num_idxs=chunk,
                                num_idxs_reg=chunk,
                                elem_size=elem_mult * MC * D,
                                transpose=True,
                                single_packet=single_packet,
                                queue_num=(2 * (g * nj + j) + k) % queues,
                            )
                        if dt8:
                            xv = (
                                xb[:]
                                .rearrange("p k a e -> p k (a e)")
                                .rearrange("p k (e m) -> p k e m", m=MC)
                            )
                            xs_t.append(xv)
                            xd_t.append(xv)
                        else:
                            xs_t.append(xs)
                            xd_t.append(xd)
                    if gather_only:
                        continue
                    for o_ in range(0, Q, SUBW):
                        xs = xs_t[o_ // chunk]
                        xd = xd_t[o_ // chunk]
                        oc = o_ % chunk
                        for mc in range(MC):
                            hts = []
                            for c in range(H // P):
                                ph = phpool.tile([P, SUBW], f32, tag=f"ph{c}")
                                if compute and dt8:
                                    nc.tensor.matmul(
                                        ph[:],
                                        lhsT=w1_sb[:, :, c * P:(c + 1) * P],
                                        rhs=xs[:, :, oc:oc + SUBW, mc],
                                        start=True, stop=True,
                                        perf_mode=drow,
                                    )
                                elif compute:
                                    nc.tensor.matmul(
                                        ph[:],
                                        lhsT=w1_sb[:, 0, c * P:(c + 1) * P],
                                        rhs=xs[:, mc, oc:oc + SUBW],
                                        start=True, stop=False,
                                    )
                                    nc.tensor.matmul(
                                        ph[:],
                                        lhsT=w1_sb[:, 1, c * P:(c + 1) * P],
                                        rhs=xd[:, mc, oc:oc + SUBW],
                                        start=False, stop=True,
                                    )
                                ht = htpool.tile([P, SUBW], f16, tag=f"ht{c}")
                                nc.scalar.activation(
                                    ht[:], ph[:], relu,
                                    bias=b1_sb[:, c:c + 1],
                                )
                                hts.append(ht)

                            py = pypool.tile([P, SUBW], f32, tag="py")
                            nc.tensor.matmul(
                                py[:], lhsT=w2_sb[:, 0, :], rhs=hts[0][:],
                                start=True, stop=False,
                            )
                            nc.tensor.matmul(
                                py[:], lhsT=w2_sb[:, 1, :], rhs=hts[1][:],
                                start=False, stop=True,
                            )
                            yo = yopool.tile([P, SUBW], f16, tag="yo")
                            nc.vector.tensor_scalar_add(yo[:], py[:],
                                                        b2_sb[:, 0:1])
                            nc.sync.dma_start(
                                y[mc, :, g * Q + o_:g * Q + o_ + SUBW],
                                yo[:],
                            )
                if gather_only:
                    # Touch y once so the output tensor is produced.
                    yo = yopool.tile([P, SUBW], f16, tag="yo")
                    nc.vector.memset(yo[:], 0.0)
                    nc.sync.dma_start(y[0, :, 0:SUBW], yo[:])

            hw_loop = os.environ.get("GNN_HW_LOOP", "1") == "1"
            if repeats > 1 and hw_loop:
                with tc.For_i(0, repeats):
                    one_pass()
            else:
                for _ in range(repeats):
                    one_pass()

    nc.compile()
    return nc


def _get_program(repeats=1):
    queues = int(os.environ.get("GNN_QUEUES", "1"))
    single_packet = os.environ.get("GNN_SINGLE_PACKET", "0") == "1"
    chunk = int(os.environ.get("GNN_CHUNK", "1024"))
    xg_bufs = int(os.environ.get("GNN_XGBUFS", "2"))
    dt8 = os.environ.get("GNN_DT8", "0") == "1"
    key = (repeats, queues, single_packet, chunk, xg_bufs, dt8)
    if key not in _CACHE:
        _CACHE[key] = _build(repeats, queues=queues,
                             single_packet=single_packet, chunk=chunk,
                             xg_bufs=xg_bufs, dt8=dt8)
    return _CACHE[key]


def _wrap_idx(flat):
    """[n*16k] int -> [128, n/16] int16, wrapped by 16, replicated 8x."""
    w = flat.reshape(-1, 16).T.astype(np.int16)      # [16, n/16]
    return np.tile(w, (8, 1))                        # [128, n/16]


def _prep_edges(src, dst):
    """Group ALL edges by (src window, dst window); split groups across cores.

    Returns (idx_arrs: list of 8 [128, idx_cols] int16, perm) where
    perm[s] = original edge position of global padded slot s (-1 = padding).
    Group g occupies global slots [g*8Q, (g+1)*8Q); core c takes the
    sub-range [g*8Q + c*Q, g*8Q + (c+1)*Q).
    """
    ws = src // WN
    wd = dst // WN
    g = ws * WC + wd
    order = np.argsort(g, kind="stable")
    counts = np.bincount(g, minlength=NG)
    if counts.max() > CORES * Q:
        raise ValueError(f"group overflow: {counts.max()} > {CORES * Q}")

    perm = np.full(GSLOT, -1, dtype=np.int64)
    src_p = np.zeros(GSLOT, dtype=np.int64)
    dst_p = np.zeros(GSLOT, dtype=np.int64)
    pos = 0
    for gi in range(NG):
        n = counts[gi]
        sel = order[pos:pos + n]
        pos += n
        base = gi * CORES * Q
        perm[base:base + n] = sel
        w_s, w_d = gi // WC, gi % WC
        src_p[base:base + n] = src[sel] - w_s * WN
        dst_p[base:base + n] = dst[sel] - w_d * WN

    idx_arrs = []
    for c in range(CORES):
        cols = []
        for gi in range(NG):
            base = gi * CORES * Q + c * Q
            cols.append(_wrap_idx(src_p[base:base + Q]))
            cols.append(_wrap_idx(dst_p[base:base + Q]))
        idx_arrs.append(np.ascontiguousarray(np.concatenate(cols, axis=1)))
    return idx_arrs, perm


def kernel(edge_index, mc_embeddings, W1, b1, W2, b2):
    nc = _get_program(1)

    edge_index = np.asarray(edge_index)
    mc_embeddings = np.asarray(mc_embeddings, dtype=np.float32)
    W1 = np.asarray(W1, dtype=np.float32)
    b1 = np.asarray(b1, dtype=np.float32)
    W2 = np.asarray(W2, dtype=np.float32)
    b2 = np.asarray(b2, dtype=np.float32)

    dt8 = getattr(nc, "_gnn_dt8", False)
    if dt8:
        f8np = mybir.dt.np(mybir.dt.float8e4)
        # Feature-major mc interleave: row n = [f0mc0, f0mc1, f1mc0, ...].
        tab = np.ascontiguousarray(
            mc_embeddings.transpose(1, 2, 0).reshape(N_NODES, MC * D)
        ).astype(f8np)
        # [d, ktile(src/dst), h] for the DoubleRow lhsT.
        w1_in = np.ascontiguousarray(
            W1.reshape(2, D, H).transpose(1, 0, 2)
        ).astype(f8np)
    else:
        # mc-interleaved fp16 node table: row n = [x0[n] | x1[n]].
        tab = np.ascontiguousarray(
            mc_embeddings.transpose(1, 0, 2).reshape(N_NODES, MC * D)
        ).astype(np.float16)
        w1_in = np.ascontiguousarray(W1.reshape(2, D, H)).astype(np.float16)
    w2_in = np.ascontiguousarray(W2.reshape(H // P, P, O)).astype(np.float16)
    b1_in = np.ascontiguousarray(b1.reshape(H // P, P)).astype(np.float32)
    b2_in = np.ascontiguousarray(b2.reshape(P, 1)).astype(np.float32)

    idx64 = edge_index.astype(np.int64)
    idx_arrs, perm = _prep_edges(idx64[0], idx64[1])
    in_maps = [
        {
            "tab": tab,
            "idx": idx_arrs[c],
            "w1": w1_in,
            "w2": w2_in,
            "b1": b1_in,
            "b2": b2_in,
        }
        for c in range(CORES)
    ]

    global _last_in_maps
    _last_in_maps = in_maps
    res = run_bass_kernel_spmd(nc, in_maps, list(range(CORES)))

    # y_all[c, mc, f, g*Q + i]  <->  global slot g*8Q + c*Q + i.
    y_all = np.stack([res.results[c]["y"] for c in range(CORES)])
    glob = (
        y_all.reshape(CORES, MC, O, NG, Q)
        .transpose(1, 2, 3, 0, 4)
        .reshape(MC, O, GSLOT)
    )
    valid = perm >= 0
    out = np.empty((MC, E_TOTAL, O), dtype=np.float32)
    out[:, perm[valid], :] = glob[:, :, valid].transpose(0, 2, 1)
    return out
